# revision 2
# baseline (speedup 1.0000x reference)
"""Handshaking kernel ('cat' type) for Trainium2, 8 NeuronCores — v2.

Math: for each upper-triangular pair (i, j>=i):
    out[b, p(i,j), :] = tanh(W1 @ h_i + W2 @ h_j + bias),  W = [W1 | W2]

Per-token projections A = seq @ W1^T + bias and C = seq @ W2^T are computed
on-device (small fp16 matmuls). The pair expansion is done with "selector"
matmuls: output rows are produced in row-major order, 128 rows per tile, and
each tile's PSUM accumulates lhsT.T @ rhs where rhs is a 32-aligned partition
slice of A/C and lhsT is a per-tile 0/1 selection matrix streamed from DRAM
(pruned to the 32-aligned k-span actually used, packed into [128,128] pages,
DMAed in 4-page chunks). Four tiles share one [128, 4*512] PSUM allocation
(4 banks) so one ScalarE Tanh instruction covers all four (PSUM -> SBUF fp16),
then a single contiguous DMA writes the 4-tile group to DRAM. The host
reassembles (transpose within groups) and upcasts to fp32.

Sharding: 8 cores = 4 batches x 2 halves of the hidden dim (H=768 -> 384).
All cores run the identical program (SPMD); only input tensors differ.
"""

import os
import sys
import numpy as np

for _p in ("/opt/trn_rl_repo", "/root/.axon_site/_ro/trn_rl_repo"):
    if _p not in sys.path:
        sys.path.insert(0, _p)

B, L, H = 4, 256, 768
HH = H // 2                  # per-core hidden slice
NPAIR = L * (L + 1) // 2     # 32896
NT = NPAIR // 128            # 257 output tiles of 128 rows
GROUP = 4                    # tiles per PSUM/ACT/DMA group
NG = (NT + GROUP - 1) // GROUP   # 65 (64 full + 1 single-tile group)
PPD = 16                     # selector pages per DMA chunk

# NOTE: accumulation groups mixing tile_position row-groups crash at runtime
# (HW-verified), so "full" K=128 blocks at (0,0) is the safe packing; fp8
# selectors (HW-verified exact for 0/1) halve the DMA cost of that choice.
SEL_FP8 = True
PACK_MODE = "full"
ACT_BATCH = True

# offset of pair (i, i) in the flattened pair dim; pair (i, j) -> OFF[i] + j - i
OFF = np.array([i * L - (i * (i - 1)) // 2 for i in range(L + 1)], dtype=np.int64)
_P = np.arange(NPAIR)
I_OF_P = np.searchsorted(OFF, _P, side="right") - 1
J_OF_P = _P - OFF[I_OF_P] + I_OF_P


def _ceil32(x):
    return -(-x // 32) * 32


def _norm_span(k0, kspan):
    """Grow (k0, kspan) minimally so k0 is a legal tile_position row for kspan
    (K<=32: any 32-mult; K<=64: {0,64}; K>64: 0). Keeps lhsT row == rhs
    partition base == tile_position[0], the semantics walrus natively derives.
    """
    if kspan <= 32:
        return k0, kspan
    if kspan <= 64:
        if k0 in (0, 64):
            return k0, kspan
        newk0 = 0 if k0 < 64 else 64
        return newk0, kspan + (k0 - newk0)
    return 0, kspan + k0


def build_schedule():
    """Per-tile selector blocks + page packing.

    Returns (tiles, npages): tiles is a list (len NT) of block lists; each
    block is a dict with fam ('A'|'C'), h (source half), k0, kspan (32-mult),
    S (kspan x 128 0/1 matrix), page. Each block sits at page row k0 (its
    source-slice partition base), so matmul lhsT/rhs partition bases match.
    """
    tiles = []
    blocks_flat = []
    for t in range(NT):
        iv = I_OF_P[128 * t : 128 * (t + 1)]
        jv = J_OF_P[128 * t : 128 * (t + 1)]
        blks = []
        for fam, vals in (("A", iv), ("C", jv)):
            for h in (0, 1):
                mask = (vals >= 128 * h) & (vals < 128 * (h + 1))
                if not mask.any():
                    continue
                kloc = vals[mask] - 128 * h
                if PACK_MODE == "full":
                    k0, kspan = 0, 128
                else:
                    k0 = int(kloc.min() // 32 * 32)
                    kspan = int(min(128 - k0, _ceil32(int(kloc.max()) + 1 - k0)))
                    k0, kspan = _norm_span(k0, kspan)
                S = np.zeros((kspan, 128), dtype=np.float32)
                S[kloc - k0, np.nonzero(mask)[0]] = 1.0
                blk = dict(fam=fam, h=h, k0=k0, kspan=kspan, S=S)
                blks.append(blk)
                blocks_flat.append(blk)
        tiles.append(blks)

    # pack blocks into [128, 128] pages at fixed row k0, order-preserving;
    # a page holds blocks with disjoint [k0, k0+kspan) intervals
    page = 0
    intervals = []
    for blk in blocks_flat:
        lo, hi = blk["k0"], blk["k0"] + blk["kspan"]
        if not all(hi <= a or lo >= b for a, b in intervals):
            page += 1
            intervals = []
        intervals.append((lo, hi))
        blk["page"] = page
    npages = page + 1
    return tiles, npages


TILES, NPAGES = build_schedule()
NCHUNK = (NPAGES + PPD - 1) // PPD


def _schedule_selfcheck():
    rng = np.random.RandomState(0)
    Aa = rng.randn(L, 8)
    Cc = rng.randn(L, 8)
    src = {("A", 0): Aa[:128], ("A", 1): Aa[128:], ("C", 0): Cc[:128], ("C", 1): Cc[128:]}
    got = np.zeros((NPAIR, 8))
    for t, blks in enumerate(TILES):
        acc = np.zeros((128, 8))
        for blk in blks:
            r = src[(blk["fam"], blk["h"])][blk["k0"] : blk["k0"] + blk["kspan"]]
            acc += blk["S"].T @ r
        got[128 * t : 128 * (t + 1)] = acc
    exp = Aa[I_OF_P] + Cc[J_OF_P]
    assert np.allclose(got, exp), "schedule self-check failed"


_schedule_selfcheck()

_CACHE = {}


def _sel_array():
    """Selector pages packed as [NCHUNK*128, PPD*128] (chunk-row-major)."""
    if "sel" in _CACHE:
        return _CACHE["sel"]
    import ml_dtypes

    dt = ml_dtypes.float8_e4m3 if SEL_FP8 else np.float16
    sel = np.zeros((NCHUNK * 128, PPD * 128), dtype=dt)
    for blks in TILES:
        for blk in blks:
            ch, pp = divmod(blk["page"], PPD)
            r0 = 128 * ch + blk["k0"]
            sel[r0 : r0 + blk["kspan"], 128 * pp : 128 * (pp + 1)] = blk["S"].astype(dt)
    _CACHE["sel"] = sel
    return sel


def _build_nc():
    import concourse.bass as bass
    import concourse.bacc as bacc
    import concourse.mybir as mybir
    import concourse.tile as tile

    f32 = mybir.dt.float32
    fp16 = mybir.dt.float16
    seldt = mybir.dt.float8e4 if SEL_FP8 else mybir.dt.float16

    nc = bacc.Bacc(None, target_bir_lowering=False, debug=False)

    # host-packed: [128, 6*X] with chunk k at cols [X*k, X*(k+1))
    seqT = nc.dram_tensor("seqT", [128, 6 * L], fp16, kind="ExternalInput")
    w1t = nc.dram_tensor("w1t", [128, 6 * HH], fp16, kind="ExternalInput")
    w2t = nc.dram_tensor("w2t", [128, 6 * HH], fp16, kind="ExternalInput")
    biasr = nc.dram_tensor("biasr", [1, HH], fp16, kind="ExternalInput")
    onesr = nc.dram_tensor("onesr", [1, 128], fp16, kind="ExternalInput")
    sel = nc.dram_tensor("sel", [NCHUNK * 128, PPD * 128], seldt, kind="ExternalInput")
    NG2 = (NG + 1) // 2
    if ACT_BATCH:
        out = nc.dram_tensor(
            "out", [NG2 * 128, 2 * GROUP * HH], fp16, kind="ExternalOutput"
        )
    else:
        out = nc.dram_tensor("out", [NT * 128, HH], fp16, kind="ExternalOutput")

    with tile.TileContext(nc) as tc:
        with (
            tc.tile_pool(name="persist", bufs=1) as pers,
            tc.tile_pool(name="selp", bufs=8) as selp,
            tc.tile_pool(name="outp", bufs=3) as outp,
        ):
            seqTp = pers.tile([128, 6 * L], fp16, tag="seqTp")
            w1p = pers.tile([128, 6 * HH], fp16, tag="w1p")
            w2p = pers.tile([128, 6 * HH], fp16, tag="w2p")
            biasr_sb = pers.tile([1, HH], fp16, tag="biasr")
            onesr_sb = pers.tile([1, 128], fp16, tag="onesr")

            nc.sync.dma_start(seqTp[:], seqT[:])
            nc.sync.dma_start(w1p[:], w1t[:])
            nc.sync.dma_start(w2p[:], w2t[:])
            nc.sync.dma_start(biasr_sb[:], biasr[:])
            nc.sync.dma_start(onesr_sb[:], onesr[:])
            seqT_sb = [seqTp[:, L * k : L * (k + 1)] for k in range(6)]
            w1_sb = [w1p[:, HH * k : HH * (k + 1)] for k in range(6)]
            w2_sb = [w2p[:, HH * k : HH * (k + 1)] for k in range(6)]

            # prime the Tanh table set early (overlaps with input DMA)
            dummy = pers.tile([1, 128], fp16, tag="dummy")
            nc.scalar.activation(
                dummy[:], onesr_sb[:], mybir.ActivationFunctionType.Tanh
            )


            # ---- projections: A = seq @ W1^T + bias, C = seq @ W2^T ----
            pre_ctx = tc.tile_pool(name="pre_ps", bufs=4, space="PSUM")
            pre_ps = pre_ctx.__enter__()
            src_sb = {}
            for fam, w_sb, add_b in (("A", w1_sb, True), ("C", w2_sb, False)):
                for h in (0, 1):
                    ps = pre_ps.tile([128, HH], f32, tag="pre")
                    for k in range(6):
                        nc.tensor.matmul(
                            ps[:],
                            lhsT=seqT_sb[k][:, 128 * h : 128 * (h + 1)],
                            rhs=w_sb[k],
                            start=(k == 0),
                            stop=(k == 5 and not add_b),
                        )
                    if add_b:
                        nc.tensor.matmul(
                            ps[:], lhsT=onesr_sb[:1, :], rhs=biasr_sb[:1, :],
                            start=False, stop=True,
                        )
                    dst = pers.tile([128, HH], fp16, tag=f"{fam}{h}", name=f"{fam}{h}")
                    nc.vector.tensor_copy(dst[:], ps[:])
                    src_sb[(fam, h)] = dst
            pre_ctx.__exit__(None, None, None)

            mm_ctx = tc.tile_pool(
                name="mm_ps", bufs=2 if ACT_BATCH else 8, space="PSUM"
            )
            mm_ps = mm_ctx.__enter__()

            # ---- main loop: selector matmuls -> batched tanh -> group DMA ----
            chunk_tiles = {}

            def get_chunk(ch):
                if ch not in chunk_tiles:
                    st = selp.tile([128, PPD * 128], seldt, tag="sel", name=f"sel{ch}")
                    nc.gpsimd.dma_start(st[:], sel[128 * ch : 128 * (ch + 1), :])
                    chunk_tiles[ch] = st
                return chunk_tiles[ch]

            def emit_mms(t, ps_out):
                blks = TILES[t]
                for bi, blk in enumerate(blks):
                    ch, pp = divmod(blk["page"], PPD)
                    st = get_chunk(ch)
                    lhsT = st[
                        blk["k0"] : blk["k0"] + blk["kspan"],
                        128 * pp : 128 * (pp + 1),
                    ]
                    rhs = src_sb[(blk["fam"], blk["h"])][
                        blk["k0"] : blk["k0"] + blk["kspan"], :
                    ]
                    nc.tensor.matmul(
                        ps_out,
                        lhsT=lhsT,
                        rhs=rhs,
                        start=(bi == 0),
                        stop=(bi == len(blks) - 1),
                        tile_position=(blk["k0"], 0),
                    )

            if ACT_BATCH:
                ot2 = None
                for g in range(NG):
                    nb = min(GROUP, NT - GROUP * g)
                    pair = g % 2
                    if pair == 0:
                        ot2 = outp.tile([128, 2, GROUP, HH], fp16, tag="ot")
                    ps = mm_ps.tile([128, GROUP, 512], f32, tag="mm")
                    for b in range(nb):
                        emit_mms(GROUP * g + b, ps[:, b, 0:HH])
                    if nb == GROUP:
                        nc.scalar.activation(
                            ot2[:, pair, :, :], ps[:, :, 0:HH],
                            mybir.ActivationFunctionType.Tanh,
                        )
                    else:
                        for b in range(nb):
                            nc.scalar.activation(
                                ot2[:, pair, b, :], ps[:, b, 0:HH],
                                mybir.ActivationFunctionType.Tanh,
                            )
                    pb = g // 2
                    if g >= NG - 3:
                        # tail: per-group DMAs so the final store starts sooner
                        cw = GROUP * HH
                        nc.sync.dma_start(
                            out[128 * pb : 128 * (pb + 1),
                                pair * cw : pair * cw + nb * HH],
                            ot2[:, pair, 0:nb, :],
                        )
                    elif pair == 1:
                        nc.sync.dma_start(
                            out[128 * pb : 128 * (pb + 1), :], ot2[:, :, :, :]
                        )
            else:
                for t in range(NT):
                    ps = mm_ps.tile([128, 512], f32, tag="mm")
                    emit_mms(t, ps[:, 0:HH])
                    ot = outp.tile([128, HH], fp16, tag="ot")
                    nc.scalar.activation(
                        ot[:], ps[:, 0:HH], mybir.ActivationFunctionType.Tanh
                    )
                    nc.sync.dma_start(out[128 * t : 128 * (t + 1), :], ot[:])

            mm_ctx.__exit__(None, None, None)

    nc.compile()
    return nc


def _get_nc():
    if "nc" not in _CACHE:
        _CACHE["nc"] = _build_nc()
    return _CACHE["nc"]


def build_in_maps(seq_hiddens, W, b):
    seq_hiddens = np.asarray(seq_hiddens, dtype=np.float32)
    W = np.asarray(W, dtype=np.float32)
    b = np.asarray(b, dtype=np.float32)
    sel = _sel_array()
    w1T = np.ascontiguousarray(W[:, :H].T)  # [H(k), H(out)]
    w2T = np.ascontiguousarray(W[:, H:].T)
    ones = np.ones((1, 128), np.float16)
    in_maps = []
    for c in range(8):
        bb, hf = divmod(c, 2)
        hs = slice(hf * HH, (hf + 1) * HH)
        in_maps.append(
            {
                "seqT": np.ascontiguousarray(
                    seq_hiddens[bb].T.astype(np.float16)
                    .reshape(6, 128, L).transpose(1, 0, 2).reshape(128, 6 * L)
                ),
                "w1t": np.ascontiguousarray(
                    w1T[:, hs].astype(np.float16)
                    .reshape(6, 128, HH).transpose(1, 0, 2).reshape(128, 6 * HH)
                ),
                "w2t": np.ascontiguousarray(
                    w2T[:, hs].astype(np.float16)
                    .reshape(6, 128, HH).transpose(1, 0, 2).reshape(128, 6 * HH)
                ),
                "biasr": np.ascontiguousarray(b[hs])[None, :].astype(np.float16),
                "onesr": ones,
                "sel": sel,
            }
        )
    return in_maps


def assemble(results):
    """results: list of 8 per-core out arrays (fp16)."""
    full = np.empty((B, NPAIR, H), np.float32)
    for c in range(8):
        bb, hf = divmod(c, 2)
        o = np.asarray(results[c])
        if ACT_BATCH:
            ng2 = (NG + 1) // 2
            o = (
                o.reshape(ng2, 128, 2, GROUP, HH)
                .transpose(0, 2, 3, 1, 4)
                .reshape(ng2 * 2 * GROUP * 128, HH)[:NPAIR]
            )
        else:
            o = o[:NPAIR]
        full[bb, :, hf * HH : (hf + 1) * HH] = o.astype(np.float32)
    return full


def kernel(seq_hiddens, W, b):
    from concourse.bass_utils import run_bass_kernel_spmd

    nc = _get_nc()
    in_maps = build_in_maps(seq_hiddens, W, b)
    res = run_bass_kernel_spmd(nc, in_maps, list(range(8)))
    return assemble([res.results[c]["out"] for c in range(8)])


if __name__ == "__main__":
    nmm = sum(len(blks) for blks in TILES)
    rows = sum(blk["kspan"] for blks in TILES for blk in blks)
    import ml_dtypes  # noqa

    selbytes = _sel_array().nbytes
    print(f"tiles={NT} groups={NG} mms={nmm} selrows={rows} pages={NPAGES} "
          f"chunks={NCHUNK} selMB={selbytes/1e6:.1f}")
    rng = np.random.RandomState(0)
    sh = rng.randn(B, L, H).astype(np.float32)
    Wv = (rng.randn(H, 2 * H) * 0.02).astype(np.float32)
    bv = np.zeros(H, np.float32)
    o = kernel(seq_hiddens=sh, W=Wv, b=bv)
    # numpy reference
    ii, jj = I_OF_P, J_OF_P
    x = np.concatenate([sh[:, ii, :], sh[:, jj, :]], axis=-1)
    exp = np.tanh(np.einsum("bpk,hk->bph", x, Wv) + bv)
    err = np.abs(o - exp).max()
    print("kernel output", o.shape, o.dtype, "absmax err", err)


# revision 3
# speedup vs baseline: 1.1789x; 1.1789x over previous
"""Handshaking kernel ('cat' type) for Trainium2, 8 NeuronCores — v2.

Math: for each upper-triangular pair (i, j>=i):
    out[b, p(i,j), :] = tanh(W1 @ h_i + W2 @ h_j + bias),  W = [W1 | W2]

Per-token projections A = seq @ W1^T + bias and C = seq @ W2^T are computed
on-device (small fp16 matmuls). The pair expansion is done with "selector"
matmuls: output rows are produced in row-major order, 128 rows per tile, and
each tile's PSUM accumulates 2-3 full-K matmuls lhsT.T @ rhs where rhs is one
of the four A/C source tiles and lhsT is a per-tile 0/1 fp8 selection matrix
([128, 128] page) streamed from DRAM in 16-page chunks on the SWDGE ring while
output stores use the sync HWDGE ring in parallel. Four tiles share one
[128, 4*512] PSUM allocation (4 banks) so one ScalarE Tanh instruction covers
all four (PSUM -> SBUF fp16), and one DMA writes two such groups (1024 rows)
contiguously to DRAM. The host reassembles (transpose within groups) and
upcasts to fp32.

Sharding: 8 cores = 4 batches x 2 halves of the hidden dim (H=768 -> 384).
All cores run the identical program (SPMD); only input tensors differ.
"""

import os
import sys
import numpy as np

for _p in ("/opt/trn_rl_repo", "/root/.axon_site/_ro/trn_rl_repo"):
    if _p not in sys.path:
        sys.path.insert(0, _p)

B, L, H = 4, 256, 768
HH = H // 2                  # per-core hidden slice
NPAIR = L * (L + 1) // 2     # 32896
NT = NPAIR // 128            # 257 output tiles of 128 rows
GROUP = 4                    # tiles per PSUM/ACT/DMA group
NG = (NT + GROUP - 1) // GROUP   # 65 (64 full + 1 single-tile group)
PPD = 16                     # selector pages per DMA chunk

# NOTE: accumulation groups mixing tile_position row-groups crash at runtime
# (HW-verified), so "full" K=128 blocks at (0,0) is the safe packing; fp8
# selectors (HW-verified exact for 0/1) halve the DMA cost of that choice.
SEL_FP8 = True
PACK_MODE = "full"
ACT_BATCH = True

# offset of pair (i, i) in the flattened pair dim; pair (i, j) -> OFF[i] + j - i
OFF = np.array([i * L - (i * (i - 1)) // 2 for i in range(L + 1)], dtype=np.int64)
_P = np.arange(NPAIR)
I_OF_P = np.searchsorted(OFF, _P, side="right") - 1
J_OF_P = _P - OFF[I_OF_P] + I_OF_P


def _ceil32(x):
    return -(-x // 32) * 32


def _norm_span(k0, kspan):
    """Grow (k0, kspan) minimally so k0 is a legal tile_position row for kspan
    (K<=32: any 32-mult; K<=64: {0,64}; K>64: 0). Keeps lhsT row == rhs
    partition base == tile_position[0], the semantics walrus natively derives.
    """
    if kspan <= 32:
        return k0, kspan
    if kspan <= 64:
        if k0 in (0, 64):
            return k0, kspan
        newk0 = 0 if k0 < 64 else 64
        return newk0, kspan + (k0 - newk0)
    return 0, kspan + k0


def build_schedule():
    """Per-tile selector blocks + page packing.

    Returns (tiles, npages): tiles is a list (len NT) of block lists; each
    block is a dict with fam ('A'|'C'), h (source half), k0, kspan (32-mult),
    S (kspan x 128 0/1 matrix), page. Each block sits at page row k0 (its
    source-slice partition base), so matmul lhsT/rhs partition bases match.
    """
    tiles = []
    blocks_flat = []
    for t in range(NT):
        iv = I_OF_P[128 * t : 128 * (t + 1)]
        jv = J_OF_P[128 * t : 128 * (t + 1)]
        blks = []
        for fam, vals in (("A", iv), ("C", jv)):
            for h in (0, 1):
                mask = (vals >= 128 * h) & (vals < 128 * (h + 1))
                if not mask.any():
                    continue
                kloc = vals[mask] - 128 * h
                if PACK_MODE == "full":
                    k0, kspan = 0, 128
                else:
                    k0 = int(kloc.min() // 32 * 32)
                    kspan = int(min(128 - k0, _ceil32(int(kloc.max()) + 1 - k0)))
                    k0, kspan = _norm_span(k0, kspan)
                S = np.zeros((kspan, 128), dtype=np.float32)
                S[kloc - k0, np.nonzero(mask)[0]] = 1.0
                blk = dict(fam=fam, h=h, k0=k0, kspan=kspan, S=S)
                blks.append(blk)
                blocks_flat.append(blk)
        tiles.append(blks)

    # pack blocks into [128, 128] pages at fixed row k0, order-preserving;
    # a page holds blocks with disjoint [k0, k0+kspan) intervals
    page = 0
    intervals = []
    for blk in blocks_flat:
        lo, hi = blk["k0"], blk["k0"] + blk["kspan"]
        if not all(hi <= a or lo >= b for a, b in intervals):
            page += 1
            intervals = []
        intervals.append((lo, hi))
        blk["page"] = page
    npages = page + 1
    return tiles, npages


TILES, NPAGES = build_schedule()
NCHUNK = (NPAGES + PPD - 1) // PPD


def _schedule_selfcheck():
    rng = np.random.RandomState(0)
    Aa = rng.randn(L, 8)
    Cc = rng.randn(L, 8)
    src = {("A", 0): Aa[:128], ("A", 1): Aa[128:], ("C", 0): Cc[:128], ("C", 1): Cc[128:]}
    got = np.zeros((NPAIR, 8))
    for t, blks in enumerate(TILES):
        acc = np.zeros((128, 8))
        for blk in blks:
            r = src[(blk["fam"], blk["h"])][blk["k0"] : blk["k0"] + blk["kspan"]]
            acc += blk["S"].T @ r
        got[128 * t : 128 * (t + 1)] = acc
    exp = Aa[I_OF_P] + Cc[J_OF_P]
    assert np.allclose(got, exp), "schedule self-check failed"


_schedule_selfcheck()

_CACHE = {}


def _sel_array():
    """Selector pages packed as [NCHUNK*128, PPD*128] (chunk-row-major)."""
    if "sel" in _CACHE:
        return _CACHE["sel"]
    import ml_dtypes

    dt = ml_dtypes.float8_e4m3 if SEL_FP8 else np.float16
    sel = np.zeros((NCHUNK * 128, PPD * 128), dtype=dt)
    for blks in TILES:
        for blk in blks:
            ch, pp = divmod(blk["page"], PPD)
            r0 = 128 * ch + blk["k0"]
            sel[r0 : r0 + blk["kspan"], 128 * pp : 128 * (pp + 1)] = blk["S"].astype(dt)
    _CACHE["sel"] = sel
    return sel


def _build_nc():
    import concourse.bass as bass
    import concourse.bacc as bacc
    import concourse.mybir as mybir
    import concourse.tile as tile

    f32 = mybir.dt.float32
    fp16 = mybir.dt.float16
    seldt = mybir.dt.float8e4 if SEL_FP8 else mybir.dt.float16

    nc = bacc.Bacc(None, target_bir_lowering=False, debug=False)

    # host-packed: [128, 6*X] with chunk k at cols [X*k, X*(k+1))
    seqT = nc.dram_tensor("seqT", [128, 6 * L], fp16, kind="ExternalInput")
    w1t = nc.dram_tensor("w1t", [128, 6 * HH], fp16, kind="ExternalInput")
    w2t = nc.dram_tensor("w2t", [128, 6 * HH], fp16, kind="ExternalInput")
    biasr = nc.dram_tensor("biasr", [1, HH], fp16, kind="ExternalInput")
    onesr = nc.dram_tensor("onesr", [1, 128], fp16, kind="ExternalInput")
    sel = nc.dram_tensor("sel", [NCHUNK * 128, PPD * 128], seldt, kind="ExternalInput")
    NG2 = (NG + 1) // 2
    if ACT_BATCH:
        out = nc.dram_tensor(
            "out", [NG2 * 128, 2 * GROUP * HH], fp16, kind="ExternalOutput"
        )
    else:
        out = nc.dram_tensor("out", [NT * 128, HH], fp16, kind="ExternalOutput")

    with tile.TileContext(nc) as tc:
        with (
            tc.tile_pool(name="persist", bufs=1) as pers,
            tc.tile_pool(name="selp", bufs=8) as selp,
            tc.tile_pool(name="outp", bufs=3) as outp,
        ):
            seqTp = pers.tile([128, 6 * L], fp16, tag="seqTp")
            w1p = pers.tile([128, 6 * HH], fp16, tag="w1p")
            w2p = pers.tile([128, 6 * HH], fp16, tag="w2p")
            biasr_sb = pers.tile([1, HH], fp16, tag="biasr")
            onesr_sb = pers.tile([1, 128], fp16, tag="onesr")

            nc.sync.dma_start(seqTp[:], seqT[:])
            nc.sync.dma_start(w1p[:], w1t[:])
            nc.sync.dma_start(w2p[:], w2t[:])
            nc.sync.dma_start(biasr_sb[:], biasr[:])
            nc.sync.dma_start(onesr_sb[:], onesr[:])
            seqT_sb = [seqTp[:, L * k : L * (k + 1)] for k in range(6)]
            w1_sb = [w1p[:, HH * k : HH * (k + 1)] for k in range(6)]
            w2_sb = [w2p[:, HH * k : HH * (k + 1)] for k in range(6)]

            # prime the Tanh table set early (overlaps with input DMA)
            dummy = pers.tile([1, 128], fp16, tag="dummy")
            nc.scalar.activation(
                dummy[:], onesr_sb[:], mybir.ActivationFunctionType.Tanh
            )


            # ---- projections: A = seq @ W1^T + bias, C = seq @ W2^T ----
            pre_ctx = tc.tile_pool(name="pre_ps", bufs=4, space="PSUM")
            pre_ps = pre_ctx.__enter__()
            src_sb = {}
            for fam, w_sb, add_b in (("A", w1_sb, True), ("C", w2_sb, False)):
                for h in (0, 1):
                    ps = pre_ps.tile([128, HH], f32, tag="pre")
                    for k in range(6):
                        nc.tensor.matmul(
                            ps[:],
                            lhsT=seqT_sb[k][:, 128 * h : 128 * (h + 1)],
                            rhs=w_sb[k],
                            start=(k == 0),
                            stop=(k == 5 and not add_b),
                        )
                    if add_b:
                        nc.tensor.matmul(
                            ps[:], lhsT=onesr_sb[:1, :], rhs=biasr_sb[:1, :],
                            start=False, stop=True,
                        )
                    dst = pers.tile([128, HH], fp16, tag=f"{fam}{h}", name=f"{fam}{h}")
                    nc.vector.tensor_copy(dst[:], ps[:])
                    src_sb[(fam, h)] = dst
            pre_ctx.__exit__(None, None, None)

            mm_ctx = tc.tile_pool(
                name="mm_ps", bufs=2 if ACT_BATCH else 8, space="PSUM"
            )
            mm_ps = mm_ctx.__enter__()

            # ---- main loop: selector matmuls -> batched tanh -> group DMA ----
            chunk_tiles = {}

            def get_chunk(ch):
                if ch not in chunk_tiles:
                    st = selp.tile([128, PPD * 128], seldt, tag="sel", name=f"sel{ch}")
                    nc.gpsimd.dma_start(st[:], sel[128 * ch : 128 * (ch + 1), :])
                    chunk_tiles[ch] = st
                return chunk_tiles[ch]

            def emit_mms(t, ps_out):
                blks = TILES[t]
                for bi, blk in enumerate(blks):
                    ch, pp = divmod(blk["page"], PPD)
                    st = get_chunk(ch)
                    lhsT = st[
                        blk["k0"] : blk["k0"] + blk["kspan"],
                        128 * pp : 128 * (pp + 1),
                    ]
                    rhs = src_sb[(blk["fam"], blk["h"])][
                        blk["k0"] : blk["k0"] + blk["kspan"], :
                    ]
                    nc.tensor.matmul(
                        ps_out,
                        lhsT=lhsT,
                        rhs=rhs,
                        start=(bi == 0),
                        stop=(bi == len(blks) - 1),
                        tile_position=(blk["k0"], 0),
                    )

            if ACT_BATCH:
                ot2 = None
                for g in range(NG):
                    nb = min(GROUP, NT - GROUP * g)
                    pair = g % 2
                    if pair == 0:
                        ot2 = outp.tile([128, 2, GROUP, HH], fp16, tag="ot")
                    ps = mm_ps.tile([128, GROUP, 512], f32, tag="mm")
                    for b in range(nb):
                        emit_mms(GROUP * g + b, ps[:, b, 0:HH])
                    if nb == GROUP:
                        nc.scalar.activation(
                            ot2[:, pair, :, :], ps[:, :, 0:HH],
                            mybir.ActivationFunctionType.Tanh,
                        )
                    else:
                        for b in range(nb):
                            nc.scalar.activation(
                                ot2[:, pair, b, :], ps[:, b, 0:HH],
                                mybir.ActivationFunctionType.Tanh,
                            )
                    pb = g // 2
                    if g >= NG - 3:
                        # tail: per-group DMAs so the final store starts sooner
                        cw = GROUP * HH
                        nc.sync.dma_start(
                            out[128 * pb : 128 * (pb + 1),
                                pair * cw : pair * cw + nb * HH],
                            ot2[:, pair, 0:nb, :],
                        )
                    elif pair == 1:
                        nc.sync.dma_start(
                            out[128 * pb : 128 * (pb + 1), :], ot2[:, :, :, :]
                        )
            else:
                for t in range(NT):
                    ps = mm_ps.tile([128, 512], f32, tag="mm")
                    emit_mms(t, ps[:, 0:HH])
                    ot = outp.tile([128, HH], fp16, tag="ot")
                    nc.scalar.activation(
                        ot[:], ps[:, 0:HH], mybir.ActivationFunctionType.Tanh
                    )
                    nc.sync.dma_start(out[128 * t : 128 * (t + 1), :], ot[:])

            mm_ctx.__exit__(None, None, None)

    nc.compile()
    return nc


def _get_nc():
    if "nc" not in _CACHE:
        _CACHE["nc"] = _build_nc()
    return _CACHE["nc"]


def build_in_maps(seq_hiddens, W, b):
    seq_hiddens = np.asarray(seq_hiddens, dtype=np.float32)
    W = np.asarray(W, dtype=np.float32)
    b = np.asarray(b, dtype=np.float32)
    sel = _sel_array()
    w1T = np.ascontiguousarray(W[:, :H].T)  # [H(k), H(out)]
    w2T = np.ascontiguousarray(W[:, H:].T)
    ones = np.ones((1, 128), np.float16)
    in_maps = []
    for c in range(8):
        bb, hf = divmod(c, 2)
        hs = slice(hf * HH, (hf + 1) * HH)
        in_maps.append(
            {
                "seqT": np.ascontiguousarray(
                    seq_hiddens[bb].T.astype(np.float16)
                    .reshape(6, 128, L).transpose(1, 0, 2).reshape(128, 6 * L)
                ),
                "w1t": np.ascontiguousarray(
                    w1T[:, hs].astype(np.float16)
                    .reshape(6, 128, HH).transpose(1, 0, 2).reshape(128, 6 * HH)
                ),
                "w2t": np.ascontiguousarray(
                    w2T[:, hs].astype(np.float16)
                    .reshape(6, 128, HH).transpose(1, 0, 2).reshape(128, 6 * HH)
                ),
                "biasr": np.ascontiguousarray(b[hs])[None, :].astype(np.float16),
                "onesr": ones,
                "sel": sel,
            }
        )
    return in_maps


def assemble(results):
    """results: list of 8 per-core out arrays (fp16)."""
    full = np.empty((B, NPAIR, H), np.float32)
    for c in range(8):
        bb, hf = divmod(c, 2)
        o = np.asarray(results[c])
        if ACT_BATCH:
            ng2 = (NG + 1) // 2
            o = (
                o.reshape(ng2, 128, 2, GROUP, HH)
                .transpose(0, 2, 3, 1, 4)
                .reshape(ng2 * 2 * GROUP * 128, HH)[:NPAIR]
            )
        else:
            o = o[:NPAIR]
        full[bb, :, hf * HH : (hf + 1) * HH] = o.astype(np.float32)
    return full


def kernel(seq_hiddens, W, b):
    from concourse.bass_utils import run_bass_kernel_spmd

    nc = _get_nc()
    in_maps = build_in_maps(seq_hiddens, W, b)
    res = run_bass_kernel_spmd(nc, in_maps, list(range(8)))
    return assemble([res.results[c]["out"] for c in range(8)])


if __name__ == "__main__":
    nmm = sum(len(blks) for blks in TILES)
    rows = sum(blk["kspan"] for blks in TILES for blk in blks)
    import ml_dtypes  # noqa

    selbytes = _sel_array().nbytes
    print(f"tiles={NT} groups={NG} mms={nmm} selrows={rows} pages={NPAGES} "
          f"chunks={NCHUNK} selMB={selbytes/1e6:.1f}")
    rng = np.random.RandomState(0)
    sh = rng.randn(B, L, H).astype(np.float32)
    Wv = (rng.randn(H, 2 * H) * 0.02).astype(np.float32)
    bv = np.zeros(H, np.float32)
    o = kernel(seq_hiddens=sh, W=Wv, b=bv)
    # numpy reference
    ii, jj = I_OF_P, J_OF_P
    x = np.concatenate([sh[:, ii, :], sh[:, jj, :]], axis=-1)
    exp = np.tanh(np.einsum("bpk,hk->bph", x, Wv) + bv)
    err = np.abs(o - exp).max()
    print("kernel output", o.shape, o.dtype, "absmax err", err)
